# revision 16
# baseline (speedup 1.0000x reference)
"""Trainium2 Bass/Tile kernel for an RNN-T Joiner:

    enc_p = encoder_out @ W_enc.T + b_enc          (N,200,512)
    dec_p = decoder_out @ W_dec.T + b_dec          (N,50,512)
    act   = tanh(enc_p[:,:,None,:] + dec_p[:,None,:,:])
    out   = act @ W_out.T + b_out                  (N,200,50,500)

Sharding: data-parallel over N=8 -- core i computes batch element i end to
end; the small weight matrices are replicated to every core.

v2 dataflow (per core):
  - the host pre-transposes every contraction operand (encT/decT/WencT/
    WdecT/WoutT, contraction dim leading) and converts them to bf16, so
    SBUF loads are straight DMAs and every matmul runs at 1 cycle/row,
  - projections: psum[j,t] = sum_e WencT.T @ encT (bf16), drained by the
    ACT engine with the per-partition bias folded in -> enc_pT/dec_pT f32,
  - acts: DVE broadcast-add (enc_pT[t] + dec_pT[u]) writes bf16 directly
    into a full-size [128, 10000] act tile per j-block; ACT applies tanh
    in place.  t-chunks grow [4,8,16,32,64,64,12] so the vocab matmul can
    start ~2us in,
  - vocab: per 256-cell double block, one 2-bank PSUM tile [128, 1024]
    (regions bank-aligned at 0/512) holding two accumulation groups of 4
    bf16 matmuls each,
  - drain: one DVE tensor_tensor per double block adds the pre-broadcast
    b_out and moves PSUM->SBUF (GPSIMD cannot read PSUM on TRN2); to
    compensate, 3 of 4 broadcast-adds run on GPSIMD,
  - output: 1MB DMA batches (4 double blocks per SBUF staging tile).
"""

import numpy as np
from contextlib import ExitStack

N, T, U = 8, 200, 50
E = J = 512
V = 500
VO = 500  # true vocab width (= V; kept for test.py compatibility)
CELLS = T * U
P = 128
KB = J // P  # 4 contraction blocks
DBLK = CELLS // 256  # 39 double blocks
TAIL = CELLS - DBLK * 256  # 16
CHUNKS = (
    [(0, 4), (4, 4), (8, 8)]
    + [(t0, 16) for t0 in range(16, 192, 16)]
    + [(192, 8)]
)
# packed input layout: one [512, PACKW] bf16 tensor, column offsets:
OFF_ENC, OFF_DEC, OFF_WENC, OFF_WDEC, OFF_WOUT = 0, 200, 256, 768, 1280
PACKW = 1784

_NC_CACHE = {}
SPLIT = 1280  # core operands (enc/dec/Wenc/Wdec) | deferred (Wout)


def _build_nc(loop_n=1, unroll=1):
    """loop_n > 1 wraps the kernel body in a tc.For_i hardware loop and
    unroll > 1 emits the body that many times inside the loop.  Every
    emission performs the complete kernel (input DMAs included) with the
    same DRAM in/out, so one dispatch executes the kernel loop_n*unroll
    times; test.py uses this to measure per-execution device time by slope
    with no host/dispatch overhead in the delta.  Input/projection buffers
    alternate between two parity sets so consecutive emissions pipeline:
    rep i+1's loads and projections overlap rep i's vocab tail."""
    import concourse.mybir as mybir
    import concourse.tile as tile
    from concourse import bacc

    f32 = mybir.dt.float32
    bf16 = mybir.dt.bfloat16
    ADD = mybir.AluOpType.add
    TANH = mybir.ActivationFunctionType.Tanh
    IDENT = mybir.ActivationFunctionType.Identity

    nc = bacc.Bacc("TRN2", target_bir_lowering=False, debug=False)

    pack_d = nc.dram_tensor("packed", [E, PACKW], bf16, kind="ExternalInput").ap()
    bias_d = nc.dram_tensor("biases_pk", [P, 2 * KB], f32, kind="ExternalInput").ap()
    bout_d = nc.dram_tensor("b_out_row", [1, V], bf16, kind="ExternalInput").ap()
    out_d = nc.dram_tensor("logits", [CELLS, V], bf16, kind="ExternalOutput").ap()

    with tile.TileContext(nc) as tc, ExitStack() as ctx:
        const = ctx.enter_context(tc.tile_pool(name="const", bufs=1))
        mm_ps = ctx.enter_context(tc.tile_pool(name="mm_ps", bufs=4, space="PSUM"))
        out_pool = ctx.enter_context(tc.tile_pool(name="outp", bufs=3))
        if loop_n > 1:
            ctx.enter_context(tc.For_i(0, loop_n, 1))

        # acts are shared across reps: block b of rep i+1 only overwrites
        # acts columns after rep i's vocab matmuls for those columns have
        # read them, which gives fine-grained cross-rep pipelining
        acts = [const.tile([P, CELLS], bf16, name=f"acts{jb}") for jb in range(KB)]
        ptiles = {}  # (name, rep parity) -> tile
        PRO_CHUNKS = CHUNKS[:2]   # emitted in the prologue (covers double 0)
        MAIN_CHUNKS = CHUNKS[2:]

        def ctile_for(par):
            def ctile(shape, dtype, name):
                key = (name, par)
                if key not in ptiles:
                    ptiles[key] = const.tile(shape, dtype, name=f"{name}_p{par}")
                return ptiles[key]
            return ctile

        def emit_chunk(t0, L, jb, enc_pT, dec_pT, addc):
            c0 = t0 * U
            C = L * U
            s = acts[jb][:, c0 : c0 + C]
            add_eng = nc.vector if addc % 8 in (0, 2, 5) else nc.gpsimd
            add_eng.tensor_tensor(
                out=s.rearrange("p (l u) -> p l u", u=U),
                in0=dec_pT[jb][:, None, :].broadcast_to([P, L, U]),
                in1=enc_pT[jb][:, t0 : t0 + L][:, :, None].broadcast_to([P, L, U]),
                op=ADD,
            )
            nc.scalar.activation(s, s, TANH)

        def prologue(rep):
            """Loads + projections + b_out broadcast + the first two
            t-chunks.  Emitted mid-way through the PREVIOUS rep's vocab
            loop so this ladder (which serializes through the ACT engine)
            overlaps the previous rep's tail instead of stalling the PE at
            the rep boundary."""
            par = rep % 2
            ctile = ctile_for(par)
            bias_sb = ctile([P, 2 * KB], f32, "bias")
            nc.sync.dma_start(bias_sb[:], bias_d[:, :])
            b_enc_sb = bias_sb[:, 0:KB]
            b_dec_sb = bias_sb[:, KB : 2 * KB]
            big = [ctile([P, PACKW], bf16, f"pack{kb}") for kb in range(KB)]
            for kb in range(KB):
                nc.sync.dma_start(
                    big[kb][:, :SPLIT], pack_d[kb * P : (kb + 1) * P, :SPLIT]
                )
            encT = [b[:, OFF_ENC : OFF_ENC + T] for b in big]
            decT = [b[:, OFF_DEC : OFF_DEC + U] for b in big]
            W_encT = [b[:, OFF_WENC : OFF_WENC + J] for b in big]
            W_decT = [b[:, OFF_WDEC : OFF_WDEC + J] for b in big]
            bout_sb = ctile([1, V], bf16, "bout")
            nc.sync.dma_start(bout_sb[:], bout_d[:, :])
            for kb in range(KB):
                nc.sync.dma_start(
                    big[kb][:, SPLIT:PACKW], pack_d[kb * P : (kb + 1) * P, SPLIT:PACKW]
                )
            ones1 = ctile([1, P], bf16, "ones")
            if rep < 2:
                nc.gpsimd.memset(ones1[:], 1.0)

            if rep == 0:
                # warm the PE clock gate during the input-DMA window
                wps = mm_ps.tile([P, 1024], f32, tag="mm", name=f"warm_ps{rep}")
                for _ in range(20):
                    nc.tensor.matmul(
                        wps[:, :P], lhsT=ones1[:], rhs=ones1[:], start=True, stop=True
                    )
                # prefetch the ACT engine's tanh table while the DMAs run
                warm = ctile([1, 8], f32, "warm")
                nc.gpsimd.memset(warm[:], 0.0)
                nc.scalar.activation(warm[:], warm[:], TANH)

            def project_jb(WT, srcT, b_sb, width, nm, jb):
                pp = mm_ps.tile([P, 1024], f32, tag="mm", name=f"{nm}_ps{jb}_{rep}")[
                    :, :width
                ]
                for kb in range(KB):
                    nc.tensor.matmul(
                        pp[:],
                        lhsT=WT[kb][:, jb * P : (jb + 1) * P],
                        rhs=srcT[kb][:],
                        start=(kb == 0),
                        stop=(kb == KB - 1),
                    )
                o = ctile([P, width], f32, f"{nm}{jb}")
                nc.scalar.activation(o[:], pp[:], IDENT, bias=b_sb[:, jb : jb + 1])
                return o

            enc_pT, dec_pT = [], []
            for jb in range(KB):
                enc_pT.append(project_jb(W_encT, encT, b_enc_sb, T, "encp", jb))
                dec_pT.append(project_jb(W_decT, decT, b_dec_sb, U, "decp", jb))

            bp = mm_ps.tile([P, 1024], f32, tag="mm", name=f"bout_ps{rep}")[:, :V]
            nc.tensor.matmul(bp[:], lhsT=ones1[:], rhs=bout_sb[:], start=True, stop=True)
            bout_rep = ctile([P, V], f32, "bout_rep")
            nc.vector.tensor_copy(bout_rep[:], bp[:])

            addc = 0
            for t0, L in PRO_CHUNKS:
                for jb in range(KB):
                    emit_chunk(t0, L, jb, enc_pT, dec_pT, addc)
                    addc += 1
            W_outT = [b[:, OFF_WOUT : OFF_WOUT + V] for b in big]
            return enc_pT, dec_pT, bout_rep, W_outT, addc

        def main(rep, pro, next_pro):
            """Remaining chunks + the vocab loop.  next_pro is called after
            vocab double PRO_AT to emit the NEXT rep's prologue."""
            enc_pT, dec_pT, bout_rep, W_outT, addc = pro
            QD = 4  # double blocks per staging tile / DMA batch
            PRO_AT = 30

            def vocab_region(reg, cells_lo, n_cells):
                for jb in range(KB):
                    nc.tensor.matmul(
                        reg,
                        lhsT=acts[jb][:, cells_lo : cells_lo + n_cells],
                        rhs=W_outT[jb][:],
                        start=(jb == 0),
                        stop=(jb == KB - 1),
                    )

            pend = []  # drained-but-not-DMAd (q, staging tile, first double)
            flush_plan = [1, 1, 2] if rep == 0 else []  # then QD

            def flush_dma():
                if not pend:
                    return
                nd = len(pend)
                ob = pend[0][1]
                d0 = pend[0][2]
                c0 = d0 * 256
                dst = out_d[c0 : c0 + nd * 256, :].rearrange("(b p) v -> p b v", p=P)
                nc.sync.dma_start(
                    dst, ob[:, : nd * 2 * V].rearrange("p (b v) -> p b v", v=V)
                )
                pend.clear()

            def emit_double(d):
                ps = mm_ps.tile([P, 1024], f32, tag="mm", name=f"ps{d}_{rep}")
                for r in range(2):
                    vocab_region(ps[:, r * 512 : r * 512 + V], d * 256 + r * P, P)
                q = len(pend)
                ob = pend[0][1] if pend else out_pool.tile(
                    [P, QD * 2 * V], bf16, tag="ob", name=f"ob{d}_{rep}"
                )
                nc.vector.tensor_tensor(
                    out=ob[:, q * 2 * V : (q + 1) * 2 * V].rearrange(
                        "p (b v) -> p b v", v=V
                    ),
                    in0=ps.rearrange("p (b q) -> p b q", q=512)[:, :, :V],
                    in1=bout_rep[:, None, :].broadcast_to([P, 2, V]),
                    op=ADD,
                )
                pend.append((q, ob, d - q))
                target = flush_plan[0] if flush_plan else QD
                if len(pend) >= target:
                    if flush_plan:
                        flush_plan.pop(0)
                    flush_dma()

            db = 0
            for t0, L in MAIN_CHUNKS:
                for jb in range(KB):
                    emit_chunk(t0, L, jb, enc_pT, dec_pT, addc)
                    addc += 1
                covered = (t0 + L) * U
                while db < DBLK and (db + 1) * 256 <= covered:
                    emit_double(db)
                    db += 1
                    if db == PRO_AT and next_pro is not None:
                        next_pro()
            flush_dma()
            if TAIL:
                ps = mm_ps.tile([P, 1024], f32, tag="mm", name=f"ps_tail{rep}")
                vocab_region(ps[:TAIL, :V], DBLK * 256, TAIL)
                obt = out_pool.tile(
                    [P, QD * 2 * V], bf16, tag="ob", name=f"ob_tail{rep}"
                )
                nc.vector.tensor_tensor(
                    out=obt[:TAIL, :V],
                    in0=ps[:TAIL, :V],
                    in1=bout_rep[:TAIL, :],
                    op=ADD,
                )
                nc.sync.dma_start(out_d[DBLK * 256 : CELLS, :], obt[:TAIL, :V])

        pros = {0: prologue(0)}

        def make_next(rep):
            if rep + 1 >= unroll:
                return None
            def emit():
                pros[rep + 1] = prologue(rep + 1)
            return emit

        for rep in range(unroll):
            main(rep, pros[rep], make_next(rep))

    nc.compile()
    return nc


def get_nc(loop_n=1, unroll=1):
    key = (loop_n, unroll)
    if key not in _NC_CACHE:
        _NC_CACHE[key] = _build_nc(loop_n, unroll)
    return _NC_CACHE[key]


def make_in_maps(inputs):
    import concourse.mybir as mybir

    bf = mybir.dt.np(mybir.dt.bfloat16)

    enc = np.asarray(inputs["encoder_out"], dtype=np.float32)
    dec = np.asarray(inputs["decoder_out"], dtype=np.float32)
    w_pack = np.zeros((E, PACKW), dtype=np.float32)
    w_pack[:, OFF_WENC : OFF_WENC + J] = np.asarray(inputs["W_enc"]).T
    w_pack[:, OFF_WDEC : OFF_WDEC + J] = np.asarray(inputs["W_dec"]).T
    w_pack[:, OFF_WOUT : OFF_WOUT + V] = np.asarray(inputs["W_out"]).T
    biases = np.empty((P, 2 * KB), dtype=np.float32)
    biases[:, 0:KB] = np.asarray(inputs["b_enc"], dtype=np.float32).reshape(KB, P).T
    biases[:, KB : 2 * KB] = (
        np.asarray(inputs["b_dec"], dtype=np.float32).reshape(KB, P).T
    )
    bout = np.asarray(inputs["b_out"], dtype=np.float32).reshape(1, V).astype(bf)
    maps = []
    for i in range(N):
        pk = w_pack.copy()
        pk[:, OFF_ENC : OFF_ENC + T] = enc[i].T
        pk[:, OFF_DEC : OFF_DEC + U] = dec[i].T
        maps.append(
            {
                "packed": np.ascontiguousarray(pk.astype(bf)),
                "biases_pk": biases,
                "b_out_row": bout,
            }
        )
    return maps


def kernel(**inputs):
    from concourse.bass_utils import run_bass_kernel_spmd

    nc = get_nc()
    in_maps = make_in_maps(inputs)
    res = run_bass_kernel_spmd(nc, in_maps, core_ids=list(range(N)))
    out = np.stack(
        [np.asarray(r["logits"], dtype=np.float32) for r in res.results], axis=0
    )
    return out.reshape(N, T, U, V)
